# revision 22
# baseline (speedup 1.0000x reference)
"""Trainium2 Bass kernel for masked-biaffine BERT self-attention.

Strategy: data-parallel over batch (16 batches / 8 cores = 2 per core).
Scores are computed TRANSPOSED (S[j,i], keys on partitions) so the additive
attention_mask is a per-partition ACT bias, softmax normalization comes free
from a ones-column in the V matmul, and probs feed the context matmul with no
transposes.

v3: software-pipelined attention at F=512 per (head, jc) step so LdWeights
hides under matmul execution:
  - PSUM: ss [128,5,512] (5 banks, single-buffered; DVE consumption staggered
    per structure pair unlocks banks for the next step), s0 [128,512] x2
    (S0 + 3 identity-matmul folds + exp, double-buffered), ctx 1 bank.
  - per step: PE computes 5 structure scores + S0; DVE mask-multiplies
    (2+2+1 ops, staggered); Pool partial-sums u pairs (f16); consumers run
    1 step later (idents+exp), ctx 2 steps later.
  - output path uses DMA-transpose (XBAR) instead of PE transposes.
  - q-side biaffine projections (qs = q @ B_s) are all computed upfront with
    block-diagonal per-head-pair [128,128] weights; QK projections per
    head-pair (M=128); V natural with ones-columns.
Host-side work is layout-only (transposes / dtype views / slicing).
"""

import sys

if "/opt/trn_rl_repo" not in sys.path:
    sys.path.insert(0, "/opt/trn_rl_repo")

import json

import numpy as np
import ml_dtypes

import concourse.bass as bass
import concourse.mybir as mybir
import concourse.tile as tile
from concourse.masks import make_identity
from concourse.bass_utils import run_bass_kernel_spmd

# ---- BIR post-pass: this walrus build allows only one sync_info.on_wait ----
# entry per instruction; hoist extras onto inserted NoOps on the same engine.
_MAXW = 1
_split_ctr = [0]


def _split_waits_json(j):
    nsplit = 0
    for fn in j.get("functions", []):
        for blk in fn.get("blocks", []):
            out = []
            for body in blk.get("instructions", []):
                si = body.get("sync_info") or {}
                ow = si.get("on_wait") or []
                if len(ow) > _MAXW:
                    extra = ow[:-_MAXW]
                    si["on_wait"] = ow[-_MAXW:]
                    while extra:
                        grp, extra = extra[:_MAXW], extra[_MAXW:]
                        _split_ctr[0] += 1
                        out.append({
                            "debug": body.get("debug", 0),
                            "engine": body["engine"],
                            "ins": [],
                            "name": f"I-waitsplit-{_split_ctr[0]}",
                            "opcode": "NoOp",
                            "outs": [],
                            "sync_info": {"on_update": [], "on_wait": grp},
                        })
                    nsplit += 1
                out.append(body)
            blk["instructions"] = out
    return nsplit


def _install_birfix():
    import concourse.bass_utils as bu
    import concourse.bass2jax as b2j

    if getattr(bu, "_waitsplit_installed", False):
        return
    orig = bu.compile_bir_kernel

    def patched(bir_json, tmpdir, neff_name="file.neff"):
        j = json.loads(bir_json)
        _split_waits_json(j)
        return orig(json.dumps(j).encode(), tmpdir, neff_name)

    bu.compile_bir_kernel = patched
    b2j.compile_bir_kernel = patched
    bu._waitsplit_installed = True


_install_birfix()

B, L, HID, H, D = 16, 512, 768, 12, 64
NS = 5
NCORES = 8
NB = B // NCORES          # batches per core
TOK = NB * L              # tokens per core
NJC = L // 128            # j-chunks per (b,h)
NP = H // 2               # head pairs
F32 = mybir.dt.float32
BF16 = mybir.dt.bfloat16
F16 = mybir.dt.float16
AF = mybir.ActivationFunctionType
OP = mybir.AluOpType

LAST_RESULT = None  # BassKernelResults of the most recent run (for test.py)


def _build_nc(ab_zero=True, debug=False):
    nc = bass.Bass()
    dbg = {}
    if debug:
        dbg["u"] = nc.dram_tensor("dbg_u", [128, NS, L], F32, kind="ExternalOutput")
        dbg["s0"] = nc.dram_tensor("dbg_s0", [128, L], F32, kind="ExternalOutput")

    # ---- DRAM I/O (per core) ----
    xt_h = nc.dram_tensor("xt", [HID, TOK], F16, kind="ExternalInput")
    wqt_h = nc.dram_tensor("wqt", [HID, HID], F16, kind="ExternalInput")
    wkt_h = nc.dram_tensor("wkt", [HID, HID], F16, kind="ExternalInput")
    wvt_h = nc.dram_tensor("wvt", [HID, HID], F16, kind="ExternalInput")
    bqt_h = nc.dram_tensor("bqt", [128, NP], F32, kind="ExternalInput")
    bkt_h = nc.dram_tensor("bkt", [128, NP], F32, kind="ExternalInput")
    bv_h = nc.dram_tensor("bv", [HID], F32, kind="ExternalInput")
    # block-diagonal per-head-pair biaffine weights [128, NP, NS, 128]
    bilibd_h = nc.dram_tensor("bilibd", [128, NP, NS, 128], BF16, kind="ExternalInput")
    absb_h = nc.dram_tensor("absb", [NS * H], F32, kind="ExternalInput")
    amt_h = nc.dram_tensor("amt", [128, NB * NJC], F32, kind="ExternalInput")
    # masks per (b): [128(jpart), NJC, NS, L] bf16
    maskt_h = nc.dram_tensor("maskt", [NB, 128, NJC, NS, L], BF16, kind="ExternalInput")
    out_h = nc.dram_tensor("out", [TOK, HID], F32, kind="ExternalOutput")

    with tile.TileContext(nc) as tc:
        with tc.tile_pool(name="pers", bufs=1) as pers:
            # persistent SBUF tensors
            qt_t = [[pers.tile([128, L], BF16, tag=f"qt{b}_{p}", name=f"qt{b}_{p}")
                     for p in range(NP)] for b in range(NB)]
            kt_t = [[pers.tile([128, L], BF16, tag=f"kt{b}_{p}", name=f"kt{b}_{p}")
                     for p in range(NP)] for b in range(NB)]
            qs_t = [[pers.tile([128, NS, L], BF16, tag=f"qs{b}_{p}", name=f"qs{b}_{p}")
                     for p in range(NP)] for b in range(NB)]
            v_t = [pers.tile([128, H * 65], F16, tag=f"v{ic}", name=f"v{ic}")
                   for ic in range(NB * NJC)]
            mask_t = [pers.tile([128, NJC, NS, L], BF16, tag=f"mask{b}", name=f"mask{b}")
                      for b in range(NB)]
            bilibd_sb = pers.tile([128, NP, NS, 128], BF16, tag="bilibd")
            absb_sb = pers.tile([128, NS * H], F32, tag="absb")
            amt_sb = pers.tile([128, NB * NJC], F32, tag="amt")
            bqt_sb = pers.tile([128, NP], F32, tag="bqt")
            bkt_sb = pers.tile([128, NP], F32, tag="bkt")
            bv_sb = pers.tile([128, HID], F32, tag="bv")
            ident = pers.tile([128, 128], F16, tag="ident")

            make_identity(nc, ident[:, :])
            nc.sync.dma_start(out=bilibd_sb[:, :, :, :], in_=bilibd_h[:, :, :, :])
            nc.sync.dma_start(out=amt_sb[:, :], in_=amt_h[:, :])
            nc.sync.dma_start(out=bqt_sb[:, :], in_=bqt_h[:, :])
            nc.sync.dma_start(out=bkt_sb[:, :], in_=bkt_h[:, :])
            for b in range(NB):
                nc.sync.dma_start(out=mask_t[b][:, :, :, :], in_=maskt_h[b, :, :, :, :])
            ab_ap = absb_h[:]
            nc.gpsimd.dma_start(
                out=absb_sb[:, :],
                in_=bass.AP(tensor=ab_ap.tensor, offset=ab_ap.offset,
                            ap=[[0, 128], [1, NS * H]]),
            )
            bv_ap = bv_h[:]
            nc.gpsimd.dma_start(
                out=bv_sb[:, :],
                in_=bass.AP(tensor=bv_ap.tensor, offset=bv_ap.offset,
                            ap=[[0, 128], [1, HID]]),
            )
            # ones columns of v_ext: preset whole tile to 1.0; projection
            # evacuations overwrite the 64 value columns of each head slot.
            for ic in range(NB * NJC):
                nc.vector.memset(v_t[ic][:, :], 1.0)

            with (
                tc.tile_pool(name="stage", bufs=1) as stb,
                tc.tile_pool(name="att", bufs=1) as att,
                tc.tile_pool(name="ss_ps", bufs=1, space="PSUM") as ssp,
                tc.tile_pool(name="s0_ps", bufs=1, space="PSUM") as s0p,
                tc.tile_pool(name="ctx_ps", bufs=1, space="PSUM") as ctxp,
            ):
                xt_sb = stb.tile([128, 6, TOK], F16, tag="xt")
                wq_sb = stb.tile([128, 6, HID], F16, tag="wq")
                wk_sb = stb.tile([128, 6, HID], F16, tag="wk")
                wv_sb = stb.tile([128, 6, HID], F16, tag="wv")
                for hc in range(6):
                    nc.sync.dma_start(out=xt_sb[:, hc, :], in_=xt_h[hc * 128:(hc + 1) * 128, :])
                    nc.sync.dma_start(out=wq_sb[:, hc, :], in_=wqt_h[hc * 128:(hc + 1) * 128, :])
                    nc.sync.dma_start(out=wk_sb[:, hc, :], in_=wkt_h[hc * 128:(hc + 1) * 128, :])
                    nc.sync.dma_start(out=wv_sb[:, hc, :], in_=wvt_h[hc * 128:(hc + 1) * 128, :])

                # ---- phase 1: QT / KT per head-pair [128, L], ACT-bias evac
                for b in range(NB):
                    for p in range(NP):
                        for (w_sb, t_sb, bias_sb) in ((wq_sb, qt_t, bqt_sb),
                                                      (wk_sb, kt_t, bkt_sb)):
                            ps = s0p.tile([128, L], F32, tag="s0", bufs=2)
                            for hc in range(6):
                                nc.tensor.matmul(
                                    ps[:, :],
                                    lhsT=w_sb[:, hc, p * 128:(p + 1) * 128],
                                    rhs=xt_sb[:, hc, b * L:(b + 1) * L],
                                    start=(hc == 0), stop=(hc == 5),
                                )
                            nc.scalar.activation(
                                t_sb[b][p][:, :], ps[:, :], AF.Identity,
                                bias=bias_sb[:, p:p + 1], scale=1.0,
                            )
                # ---- phase 1.5: all qs upfront (block-diag pair weights);
                # evacuations split between ACT and DVE.
                for b in range(NB):
                    for p in range(NP):
                        for s in range(NS):
                            ps = s0p.tile([128, L], F32, tag="s0", bufs=2)
                            nc.tensor.matmul(
                                ps[:, :],
                                lhsT=bilibd_sb[:, p, s, :],
                                rhs=qt_t[b][p][:, :],
                                start=True, stop=True,
                            )
                            if s % 2 == 0:
                                nc.scalar.copy(qs_t[b][p][:, s, :], ps[:, :])
                            else:
                                nc.vector.tensor_copy(qs_t[b][p][:, s, :], ps[:, :])
                # ---- phase 1.75: V natural [tok, o] with bias add (DVE);
                # cols 0:512 in bank slot 0, cols 512:768 in slot 1.
                for ic in range(NB * NJC):
                    ps = ssp.tile([128, NS, L], F32, tag="ss", bufs=1)
                    for hc in range(6):
                        nc.tensor.matmul(
                            ps[:, 0, :],
                            lhsT=xt_sb[:, hc, ic * 128:(ic + 1) * 128],
                            rhs=wv_sb[:, hc, 0:512],
                            start=(hc == 0), stop=(hc == 5),
                        )
                    for hc in range(6):
                        nc.tensor.matmul(
                            ps[:, 1, 0:256],
                            lhsT=xt_sb[:, hc, ic * 128:(ic + 1) * 128],
                            rhs=wv_sb[:, hc, 512:768],
                            start=(hc == 0), stop=(hc == 5),
                        )
                    vv = v_t[ic][:, :].rearrange("p (h e) -> p h e", e=65)
                    nc.vector.tensor_add(
                        vv[:, 0:8, 0:64],
                        ps[:, 0, :].rearrange("p (h q) -> p h q", q=64),
                        bv_sb[:, 0:512].rearrange("p (h q) -> p h q", q=64),
                    )
                    nc.vector.tensor_add(
                        vv[:, 8:12, 0:64],
                        ps[:, 1, 0:256].rearrange("p (h q) -> p h q", q=64),
                        bv_sb[:, 512:768].rearrange("p (h q) -> p h q", q=64),
                    )

                # ---- phase 2: attention, F=512 per (b, p, hp, jc) step ----
                steps = [(b, p, hp, jc)
                         for b in range(NB) for p in range(NP) for hp in range(2)
                         for jc in range(NJC)]
                nsteps = len(steps)
                sched = [[] for _ in range(nsteps + 4)]
                ctx_ps_by_head = {}

                def emit_step(k, b, p, hp, jc):
                    bp = 64 * hp
                    head = 2 * p + hp
                    kt_j = kt_t[b][p][bp:bp + 64, jc * 128:(jc + 1) * 128]

                    ss = ssp.tile([128, NS, L], F32, tag="ss", bufs=1)
                    s0 = s0p.tile([128, L], F32, tag="s0", bufs=2)
                    u = att.tile([128, NS, L], F16, tag="u", bufs=2)
                    a = att.tile([128, 2, L], F16, tag="a", bufs=2)
                    pr = att.tile([128, L], F16, tag="pr", bufs=3)

                    # PE: 5 structure scores (own banks), then S0 last
                    for s in range(NS):
                        nc.tensor.matmul(ss[:, s, :], lhsT=kt_j,
                                         rhs=qs_t[b][p][bp:bp + 64, s, :],
                                         start=True, stop=True,
                                         skip_group_check=True)
                    nc.tensor.matmul(s0[:, :], lhsT=kt_j,
                                     rhs=qt_t[b][p][bp:bp + 64, :],
                                     start=True, stop=False, skip_group_check=True)
                    # DVE: mask-combine, staggered 2+2+1
                    mk = mask_t[b][:, jc, :, :]
                    if ab_zero:
                        nc.vector.tensor_tensor(
                            u[:, 0:2, :], ss[:, 0:2, :], mk[:, 0:2, :], OP.mult)
                        nc.vector.tensor_tensor(
                            u[:, 2:4, :], ss[:, 2:4, :], mk[:, 2:4, :], OP.mult)
                        nc.vector.tensor_tensor(
                            u[:, 4, :], ss[:, 4, :], mk[:, 4, :], OP.mult)
                    else:
                        for s in range(NS):
                            nc.vector.scalar_tensor_tensor(
                                u[:, s, :], ss[:, s, :],
                                absb_sb[:, s * H + head:s * H + head + 1],
                                mk[:, s, :], OP.add, OP.mult)
                    # Pool: partial sums (f16)
                    nc.gpsimd.tensor_tensor(a[:, 0, :], u[:, 0, :], u[:, 1, :], OP.add)
                    nc.gpsimd.tensor_tensor(a[:, 1, :], u[:, 2, :], u[:, 3, :], OP.add)

                    def cons1(k=k, b=b, jc=jc, s0=s0, u=u, a=a, pr=pr):
                        # PE: fold masked biases into s0, close group; ACT exp
                        nc.tensor.matmul(s0[:, :], lhsT=ident[:, :], rhs=a[:, 0, :],
                                         start=False, stop=False, skip_group_check=True)
                        nc.tensor.matmul(s0[:, :], lhsT=ident[:, :], rhs=a[:, 1, :],
                                         start=False, stop=False, skip_group_check=True)
                        nc.tensor.matmul(s0[:, :], lhsT=ident[:, :], rhs=u[:, 4, :],
                                         start=False, stop=True, skip_group_check=True)
                        nc.scalar.activation(
                            pr[:, :], s0[:, :], AF.Exp,
                            bias=amt_sb[:, b * NJC + jc:b * NJC + jc + 1],
                            scale=0.125,
                        )
                        if debug and k == 0:
                            du = att.tile([128, NS, L], F16, tag="du", name="du")
                            nc.scalar.copy(du[:, :, :], u[:, :, :])
                            nc.gpsimd.dma_start(out=dbg["u"][:, :, :], in_=du[:, :, :])
                            ds0 = att.tile([128, L], F16, tag="ds0", name="ds0")
                            nc.scalar.copy(ds0[:, :], s0[:, :])
                            nc.gpsimd.dma_start(out=dbg["s0"][:, :], in_=ds0[:, :])

                    def cons2(k=k, b=b, p=p, hp=hp, jc=jc, head=head, pr=pr):
                        key = (b, head)
                        if key not in ctx_ps_by_head:
                            ctx_ps_by_head[key] = ctxp.tile(
                                [128, L], F32, tag="ctx", bufs=1, name=f"ctx{b}_{head}")
                        cps = ctx_ps_by_head[key]
                        nc.tensor.matmul(
                            cps[0:65, :],
                            lhsT=v_t[b * NJC + jc][:, head * 65:(head + 1) * 65],
                            rhs=pr[:, :],
                            start=(jc == 0), stop=(jc == NJC - 1),
                            skip_group_check=True,
                        )
                        if jc == NJC - 1:
                            ct = att.tile([128, L], F16, tag="ct", bufs=2)
                            # rows 65:96 are ballast read by the sums
                            # DMA-transpose; keep them initialized.
                            nc.gpsimd.memset(ct[64:96, :], 1.0)
                            nc.scalar.copy(ct[0:65, :], ctx_ps_by_head.pop(key)[0:65, :])
                            sched[min(k + 2, nsteps + 3)].append(
                                lambda b=b, head=head, ct=ct: finalize_head(b, head, ct))

                    sched[k + 1].append(cons1)
                    sched[k + 2].append(cons2)

                def finalize_head(b, head, ct):
                    # DMA-transpose ct[0:64] per j-chunk; scatter the sums row
                    cot = att.tile([128, NJC, D], F16, tag="cot", bufs=2)
                    sums = att.tile([128, NJC, 32], F16, tag="sums", bufs=1)
                    for icb in range(NJC):
                        nc.sync.dma_start_transpose(
                            cot[:, icb, :], ct[0:64, icb * 128:(icb + 1) * 128])
                        # row 64 holds the softmax denominators; rows 65:96
                        # are ballast to satisfy the 32-row XBAR constraint.
                        nc.sync.dma_start_transpose(
                            sums[:, icb, :], ct[64:96, icb * 128:(icb + 1) * 128])
                    rec = att.tile([128, NJC], F32, tag="rec", bufs=2)
                    nc.vector.reciprocal(rec[:, :], sums[:, :, 0])
                    co = att.tile([128, NJC, D], F32, tag="co", bufs=1)
                    for icb in range(NJC):
                        nc.scalar.activation(
                            co[:, icb, :], cot[:, icb, :], AF.Copy,
                            bias=0.0, scale=rec[:, icb:icb + 1],
                        )
                    dst = out_h[b * L:(b + 1) * L, head * D:(head + 1) * D].rearrange(
                        "(c p) d -> p c d", c=NJC)
                    nc.sync.dma_start(out=dst, in_=co[:, :, :])

                for k, st in enumerate(steps):
                    for fn in sched[k]:
                        fn()
                    sched[k] = []
                    emit_step(k, *st)
                for k in range(nsteps, nsteps + 4):
                    for fn in sched[k]:
                        fn()
                    sched[k] = []
    return nc


_NC = {}


def _get_nc(ab_zero=True):
    if ab_zero not in _NC:
        _NC[ab_zero] = _build_nc(ab_zero)
    return _NC[ab_zero]


def _prep_inputs(inputs):
    hs = np.asarray(inputs["hidden_states"], np.float32)
    am = np.asarray(inputs["attention_mask"], np.float32).reshape(B, L)
    sm = np.asarray(inputs["structure_mask"], np.float32)
    Wq = np.asarray(inputs["Wq"], np.float32)
    Wk = np.asarray(inputs["Wk"], np.float32)
    Wv = np.asarray(inputs["Wv"], np.float32)
    bq = np.asarray(inputs["bq"], np.float32)
    bk = np.asarray(inputs["bk"], np.float32)
    bv = np.asarray(inputs["bv"], np.float32)
    bili = np.asarray(inputs["bili"], np.float32)
    absb = np.asarray(inputs["abs_bias"], np.float32)

    # block-diagonal per-head-pair biaffine weights [128, NP, NS, 128]
    bilibd = np.zeros((128, NP, NS, 128), np.float32)
    for p in range(NP):
        for s in range(NS):
            bilibd[0:64, p, s, 0:64] = bili[s, 2 * p]
            bilibd[64:128, p, s, 64:128] = bili[s, 2 * p + 1]

    shared = {
        "wqt": np.ascontiguousarray(Wq.T).astype(np.float16),
        "wkt": np.ascontiguousarray(Wk.T).astype(np.float16),
        "wvt": np.ascontiguousarray(Wv.T).astype(np.float16),
        "bqt": np.ascontiguousarray(bq.reshape(NP, 128).T),
        "bkt": np.ascontiguousarray(bk.reshape(NP, 128).T),
        "bv": bv,
        "bilibd": bilibd.astype(ml_dtypes.bfloat16),
        "absb": np.ascontiguousarray(absb.reshape(NS * H)),
    }
    in_maps = []
    for c in range(NCORES):
        b0 = c * NB
        x = hs[b0:b0 + NB].reshape(TOK, HID)
        amc = am[b0:b0 + NB]  # [NB, L]
        # -10: constant logit shift (softmax-invariant) keeping exp() and the
        # row sums inside fp16 range without a max-reduction pass.
        amt = np.ascontiguousarray(
            amc.reshape(NB, NJC, 128).transpose(2, 0, 1)).reshape(128, NB * NJC) - 10.0
        mk = sm[:, b0:b0 + NB, 0]  # [NS, NB, L(i), L(j)]
        mkt = np.ascontiguousarray(mk.transpose(1, 3, 0, 2))  # [NB, j, NS, i]
        # -> [NB, NJC, 128, NS, L] -> [NB, 128, NJC, NS, L]
        mkt = mkt.reshape(NB, NJC, 128, NS, L).transpose(0, 2, 1, 3, 4)
        mkt = np.ascontiguousarray(mkt).astype(ml_dtypes.bfloat16)
        in_maps.append(dict(
            xt=np.ascontiguousarray(x.T).astype(np.float16), amt=amt, maskt=mkt, **shared))
    return in_maps


def kernel(**inputs):
    global LAST_RESULT
    ab_zero = bool(np.all(np.asarray(inputs["abs_bias"]) == 0.0))
    nc = _get_nc(ab_zero)
    in_maps = _prep_inputs(inputs)
    import os
    trace = bool(os.environ.get("BASS_TRACE"))
    LAST_RESULT = run_bass_kernel_spmd(
        nc, in_maps, core_ids=list(range(NCORES)), trace=trace)
    outs = [r["out"] for r in LAST_RESULT.results]
    return np.concatenate(outs, axis=0).reshape(B, L, HID)
